# revision 37
# baseline (speedup 1.0000x reference)
"""Two-layer GCN feature extractor on 8 Trainium2 NeuronCores.

Strategy (self-contained; all shapes hardcoded for the target problem):
  * Nodes are sharded across 8 cores (6250 rows each).  Each core:
      1. transforms its own rows: h = dinv * (x_own @ W)         (PE)
      2. AllGather -> replicated h table [N, F] in DRAM          (collective)
      3. gathers per-edge source rows with dma_gather            (SWDGE DMA)
      4. segment-sums messages into its destination rows via
         one-hot selection matmuls accumulated in PSUM, with the
         SLAB as the stationary operand so the accumulator is the
         transposed output py_T[F, dst]                          (PE + DVE)
      5. y_T = relu(dinv_col * py_T + b)  (layer 1); layer 2 output
         stays transposed and is de-transposed (+bias) on the host.
  * Performance notes vs the naive pipeline:
      - gathers sized at half the SWDGE ring (512 descriptors) so the
        Q7 descriptor generator never blocks on a full ring;
      - all scales/relu/casts run on the Scalar (ACT) engine, keeping
        the DVE out of 2-port perf mode (which would starve SWDGE
        descriptor writes through the shared SBUF port);
      - x is loaded via one HWDGE dma-transpose; both layers'
        transforms are transpose-free bf16 matmuls;
      - each AllGather is split into two zone-halves (local rows
        [0,3200) and [3200,6250)); zone-A gathers start as soon as
        AG-a lands, and layer-2's AG-a is triggered mid-way through
        layer-1's gather phase so it fully overlaps;
      - self-loop messages never hit the gather path: they are added
        per dst block by one identity matmul from the resident
        transform output.
  * Graph preprocessing (edge partitioning by destination, sorting,
    degree counting, int16 index stream layout) happens on host with
    numpy; all floating-point math runs on device.
  * dma_gather needs int16 indices; the zone split keeps both gather
    tables (8*3200 and 8*3050 rows) within int16 range.
"""

import math
import os
from contextlib import ExitStack

import numpy as np

os.environ.setdefault("MYCRO_LOCAL_CACHE", "1")

# ----------------------------------------------------------------------------
# configuration
# ----------------------------------------------------------------------------


def make_cfg(
    N=50000,
    F=128,
    ncores=8,
    split=3200,  # zone-A local rows per core (AG split point, mult of 128)
    gchunks=8,
    table_bf16=True,
    nqueues=4,
    selb=32,
    single_packet=True,
    dma_scratch=16384,
    gbufs=6,
):
    assert N % ncores == 0
    rows = N // ncores
    nblk = math.ceil(rows / 128)
    return dict(
        N=N,
        F=F,
        ncores=ncores,
        split=split,
        rows=rows,
        nblk=nblk,
        last_rows=rows - (nblk - 1) * 128,
        gchunks=gchunks,
        table_bf16=table_bf16,
        nqueues=nqueues,
        selb=selb,
        single_packet=single_packet,
        dma_scratch=dma_scratch,
        gbufs=gbufs,
    )


FULL_CFG = make_cfg()


# ----------------------------------------------------------------------------
# host-side graph preprocessing
# ----------------------------------------------------------------------------


def preprocess(edge_index, cfg):
    """Partition edges by destination core, sort by (dst block, src half),
    pad each (block, half) run to a multiple of 128, and lay out index /
    dst-local streams in the formats dma_gather and the kernel expect.

    Returns (sched, per_core, deg) where sched = (c_lo, c_hi) chunk counts
    per block (uniform across cores).
    """
    N, ncores, rows, nblk, split = (
        cfg["N"],
        cfg["ncores"],
        cfg["rows"],
        cfg["nblk"],
        cfg["split"],
    )

    src = np.asarray(edge_index[0], dtype=np.int64)
    dst = np.asarray(edge_index[1], dtype=np.int64)

    # self-loops contribute to deg but are served on-device by a per-block
    # identity matmul from the resident transform output (no gather descs)
    deg = (np.bincount(dst, minlength=N) + 1).astype(np.float32)

    core_of = dst // rows
    per_core_raw = []
    counts = np.zeros((ncores, nblk, 2), dtype=np.int64)
    for k in range(ncores):
        m = core_of == k
        s_k = src[m]
        d_k = dst[m] - k * rows
        blk = d_k >> 7
        dl = (d_k & 127).astype(np.float32)
        l_k = s_k % rows
        half = (l_k >= split).astype(np.int64)
        order = np.lexsort((s_k, half, blk))
        s_k, dl, blk, half = s_k[order], dl[order], blk[order], half[order]
        c = np.bincount(blk * 2 + half, minlength=nblk * 2).reshape(nblk, 2)
        counts[k] = c
        per_core_raw.append((s_k, dl, c))

    cdiv = lambda a, b: -(-a // b)
    c_lo = [int(max(cdiv(counts[k, b, 0], 128) for k in range(ncores))) for b in range(nblk)]
    c_hi = [int(max(cdiv(counts[k, b, 1], 128) for k in range(ncores))) for b in range(nblk)]
    S_lo = sum(c_lo) * 128
    S_hi = sum(c_hi) * 128

    per_core = []
    for k in range(ncores):
        s_k, dl_k, c = per_core_raw[k]
        # per-(block, half) start offsets into the sorted arrays
        starts = np.concatenate([[0], np.cumsum(c.reshape(-1))])
        idx_lo = np.zeros(S_lo, np.int16)
        dst_lo = np.full(S_lo, -1.0, np.float32)
        idx_hi = np.zeros(S_hi, np.int16)
        dst_hi = np.full(S_hi, -1.0, np.float32)
        plo = phi = 0
        for b in range(nblk):
            n0 = int(c[b, 0])
            o0 = int(starts[b * 2])
            sa = s_k[o0 : o0 + n0]
            idx_lo[plo : plo + n0] = (
                (sa // rows) * split + sa % rows
            ).astype(np.int16)
            dst_lo[plo : plo + n0] = dl_k[o0 : o0 + n0]
            plo += c_lo[b] * 128
            n1 = int(c[b, 1])
            o1 = int(starts[b * 2 + 1])
            sb = s_k[o1 : o1 + n1]
            idx_hi[phi : phi + n1] = (
                (sb // rows) * (rows - split) + sb % rows - split
            ).astype(np.int16)
            dst_hi[phi : phi + n1] = dl_k[o1 : o1 + n1]
            phi += c_hi[b] * 128

        def arrange_idx(a):  # logical i -> sbuf[i % 16, i // 16], tiled to 128 parts
            if a.size == 0:
                return np.zeros((128, 0), np.int16)
            return np.tile(np.ascontiguousarray(a.reshape(-1, 16).T), (8, 1))

        def arrange_dl(a):  # logical i -> sbuf[i % 128, i // 128]
            if a.size == 0:
                return np.zeros((128, 0), np.float32)
            return np.ascontiguousarray(a.reshape(-1, 128).T)

        per_core.append(
            dict(
                idx_lo=arrange_idx(idx_lo),
                idx_hi=arrange_idx(idx_hi),
                dl_lo=arrange_dl(dst_lo),
                dl_hi=arrange_dl(dst_hi),
            )
        )

    return (tuple(c_lo), tuple(c_hi)), per_core, deg


# ----------------------------------------------------------------------------
# bass program
# ----------------------------------------------------------------------------

_PROGRAM_CACHE = {}


def build_program(cfg, sched):
    import concourse.bacc as bacc
    import concourse.bass as bass
    import concourse.mybir as mybir
    import concourse.tile as tile

    c_lo, c_hi = sched
    N, F, ncores, split = cfg["N"], cfg["F"], cfg["ncores"], cfg["split"]
    rows, nblk, last_rows = cfg["rows"], cfg["nblk"], cfg["last_rows"]
    gchunks = cfg["gchunks"]
    S_lo, S_hi = sum(c_lo) * 128, sum(c_hi) * 128

    f32 = mybir.dt.float32
    i16 = mybir.dt.int16
    TD = mybir.dt.bfloat16 if cfg["table_bf16"] else f32
    nq = cfg.get("nqueues", 4)
    selb = cfg.get("selb", 32)
    mul = mybir.AluOpType.mult
    eq = mybir.AluOpType.is_equal
    COPY = mybir.ActivationFunctionType.Copy
    RELU = mybir.ActivationFunctionType.Relu

    nc = bacc.Bacc(
        "TRN2",
        target_bir_lowering=False,
        debug=False,
        enable_asserts=False,
        num_devices=ncores,
        num_swdge_queues=nq,
        dynamic_dma_scratch_size=cfg.get("dma_scratch", 16384),
    )

    x_ownd = nc.dram_tensor("x_own", [nblk * 128, F], TD, kind="ExternalInput")
    W1d = nc.dram_tensor("W1", [F, F], TD, kind="ExternalInput")
    W2d = nc.dram_tensor("W2", [F, F], TD, kind="ExternalInput")
    b1d = nc.dram_tensor("b1_col", [128, 1], f32, kind="ExternalInput")
    dinvrd = nc.dram_tensor("dinv_row", [128, nblk], f32, kind="ExternalInput")
    dinvbd = nc.dram_tensor("dinv_bc", [128, nblk * 128], TD, kind="ExternalInput")
    iotad = nc.dram_tensor("iota", [128, 128], TD, kind="ExternalInput")
    identd = nc.dram_tensor("ident", [128, 128], TD, kind="ExternalInput")
    ixlod = ixhid = dllod = dlhid = None
    if S_lo:
        ixlod = nc.dram_tensor("idx_lo", [128, S_lo // 16], i16, kind="ExternalInput")
        dllod = nc.dram_tensor("dl_lo", [128, S_lo // 128], TD, kind="ExternalInput")
    if S_hi:
        ixhid = nc.dram_tensor("idx_hi", [128, S_hi // 16], i16, kind="ExternalInput")
        dlhid = nc.dram_tensor("dl_hi", [128, S_hi // 128], TD, kind="ExternalInput")
    # transposed output: [F, rows]; host transposes back and adds b2
    yout = nc.dram_tensor("y_out", [128, rows], f32, kind="ExternalOutput")

    RA, RB = split, rows - split  # zone-A/zone-B local rows (AG split)
    h_stage = [nc.dram_tensor(f"h_stage{i}", [rows, F], TD) for i in (1, 2)]
    h_fullA = [
        nc.dram_tensor(f"h_fullA{i}", [ncores * RA, F], TD, addr_space="Shared")
        for i in (1, 2)
    ]
    h_fullB = [
        nc.dram_tensor(f"h_fullB{i}", [ncores * RB, F], TD, addr_space="Shared")
        for i in (1, 2)
    ]

    with tile.TileContext(nc) as tc, ExitStack() as ctx:
        const = ctx.enter_context(tc.tile_pool(name="const", bufs=1))
        xf = ctx.enter_context(tc.tile_pool(name="xf", bufs=3))
        xfp = ctx.enter_context(tc.tile_pool(name="xfp", bufs=2, space="PSUM"))
        gps = ctx.enter_context(tc.tile_pool(name="gps", bufs=3, space="PSUM"))
        gb = cfg.get("gbufs", 6)
        glo = ctx.enter_context(tc.tile_pool(name="glo", bufs=gb))
        ghi = ctx.enter_context(tc.tile_pool(name="ghi", bufs=gb))
        selp = ctx.enter_context(tc.tile_pool(name="selp", bufs=3))
        outp = ctx.enter_context(tc.tile_pool(name="outp", bufs=3))

        # ---- constants ---------------------------------------------------
        # big gather-side constants load on the ACT HWDGE queue so they don't
        # delay the transform/h_stage traffic on the sync queue
        def load_const(dram, shape, dtype, eng=None):
            t = const.tile(shape, dtype, tag=f"c_{dram.name}")
            (eng or nc.sync).dma_start(t[:], dram[:])
            return t

        W1s = load_const(W1d, [F, F], TD)
        W2s = load_const(W2d, [F, F], TD)
        b1s = load_const(b1d, [128, 1], f32)
        dinvr = load_const(dinvrd, [128, nblk], f32)
        dinvb = load_const(dinvbd, [128, nblk * 128], TD, nc.scalar)
        iota = load_const(iotad, [128, 128], TD, nc.scalar)
        idents = load_const(identd, [128, 128], TD, nc.scalar)
        ixlo = load_const(ixlod, [128, S_lo // 16], i16, nc.scalar) if S_lo else None
        dllo = load_const(dllod, [128, S_lo // 128], TD, nc.scalar) if S_lo else None
        ixhi = load_const(ixhid, [128, S_hi // 16], i16, nc.scalar) if S_hi else None
        dlhi = load_const(dlhid, [128, S_hi // 128], TD, nc.scalar) if S_hi else None

        # x transposed into SBUF via one HWDGE xbar transpose
        xT = const.tile([128, nblk * 128], TD)
        nc.sync.dma_start_transpose(xT[:], x_ownd[:])

        y1T = const.tile([128, nblk * 128], TD)  # layer-1 output, [F, nodes]
        # per-layer transform outputs kept resident: serve the h_stage DMA
        # and the self-loop contribution (identity matmul, no gather descs)
        hown = [
            const.tile([128, nblk * 128], TD, name=f"hown{i}") for i in (1, 2)
        ]

        cutblk = RA // 128  # zone-A spans blocks [0, cutblk)

        # ---- transform: h_stage = dinv * (rows @ W) ----------------------
        def transform_blocks(src_tile, W_s, ho, t0, t1):
            for t in range(t0, t1):
                ph = xfp.tile([128, F], f32)
                nc.tensor.matmul(
                    ph[:],
                    lhsT=src_tile[:, t * 128 : (t + 1) * 128],
                    rhs=W_s[:],
                    start=True,
                    stop=True,
                )
                hs = ho[:, t * 128 : (t + 1) * 128]
                nc.scalar.activation(hs, ph[:], COPY, scale=dinvr[:, t : t + 1])

        def stage_zone_a(stage, ho, hfA):
            nc.sync.dma_start(
                stage[0:RA, :].rearrange("(t p) f -> p t f", p=128),
                ho[:, 0:RA].rearrange("p (t f) -> p t f", f=128),
            )
            nc.gpsimd.collective_compute(
                "AllGather",
                mybir.AluOpType.bypass,
                replica_groups=[list(range(ncores))],
                ins=[stage[0:RA, :]],
                outs=[hfA[:]],
            )

        def stage_zone_b(stage, ho, hfB):
            nb1 = nblk - 1
            nc.sync.dma_start(
                stage[RA : nb1 * 128, :].rearrange("(t p) f -> p t f", p=128),
                ho[:, RA : nb1 * 128].rearrange("p (t f) -> p t f", f=128),
            )
            nc.sync.dma_start(
                stage[nb1 * 128 :, :], ho[:last_rows, nb1 * 128 :]
            )
            nc.gpsimd.collective_compute(
                "AllGather",
                mybir.AluOpType.bypass,
                replica_groups=[list(range(ncores))],
                ins=[stage[RA:rows, :]],
                outs=[hfB[:]],
            )

        # ---- gather + segment-sum reduce ---------------------------------
        qctr = [0]

        class Stream:
            def __init__(self, idx, dl, view, total_chunks, pool):
                self.idx, self.dl, self.view, self.pool = idx, dl, view, pool
                self.total = total_chunks
                self.pos = 0  # chunks consumed
                self.issued = 0  # chunks covered by issued gathers
                self.pend = []  # (base, n, slab) in issue order

            def issue_one(self):
                """Issue the gather for the next uncovered slab (prefetch)."""
                if self.issued >= self.total:
                    return
                n = min(gchunks, self.total - self.issued)
                nidx = n * 128
                slab = self.pool.tile([128, n, F], TD, tag="slab")
                nc.gpsimd.dma_gather(
                    slab[:],
                    self.view,
                    self.idx[:, self.issued * 8 : self.issued * 8 + nidx // 16],
                    nidx,
                    nidx,
                    F,
                    queue_num=qctr[0] % nq,
                    single_packet=cfg.get("single_packet", True),
                )
                qctr[0] += 1
                self.pend.append((self.issued, n, slab))
                self.issued += n

            def chunk(self):
                """Return (slab_tile, column) for the chunk at self.pos."""
                while self.pend and self.pos >= self.pend[0][0] + self.pend[0][1]:
                    self.pend.pop(0)
                    self.issue_one()  # keep the prefetch depth topped up
                if self.issued <= self.pos:
                    self.issue_one()
                base, n, slab = self.pend[0]
                col = self.pos - base
                self.pos += 1
                return slab, col

        def gather_reduce(hfA, hfB, finalize, ho, mid_cb=None, mid_at=None):
            streams = []
            if S_lo:
                streams.append(
                    (c_lo, Stream(ixlo, dllo, hfA[:], S_lo // 128, glo))
                )
            if S_hi:
                streams.append(
                    (c_hi, Stream(ixhi, dlhi, hfB[:], S_hi // 128, ghi))
                )
            # prefetch the zone-A stream: its table lands first (AG-a), and
            # issuing these ahead keeps the Q7 busy while AG-b is in flight
            if S_lo:
                for _ in range(gb - 1):
                    streams[0][1].issue_one()
            for b in range(nblk):
                py = gps.tile([128, F], f32)  # transposed accum: [F, dst]
                nch = sum(c[b] for c, _ in streams) + 1
                # self-loop contribution from the resident transform output
                nc.tensor.matmul(
                    py[:],
                    lhsT=ho[:, b * 128 : (b + 1) * 128],
                    rhs=idents[:],
                    start=True,
                    stop=(nch == 1),
                )
                i = 1
                for c, st in streams:
                    done = 0
                    while done < c[b]:
                        g = min(selb, c[b] - done)
                        p0 = st.pos
                        sel = selp.tile([128, selb, 128], TD)
                        nc.vector.tensor_tensor(
                            out=sel[:, :g, :],
                            in0=st.dl[:, p0 : p0 + g].to_broadcast([128, g, 128]),
                            in1=iota[:, None, :].to_broadcast([128, g, 128]),
                            op=eq,
                        )
                        for j in range(g):
                            slab, col = st.chunk()
                            nc.tensor.matmul(
                                py[:],
                                lhsT=slab[:, col, :],
                                rhs=sel[:, j, :],
                                start=(i == 0),
                                stop=(i == nch - 1),
                            )
                            i += 1
                        done += g
                finalize(b, py)
                if mid_cb is not None and b == mid_at:
                    mid_cb()

        # ---- layer 1 -----------------------------------------------------
        transform_blocks(xT, W1s, hown[0], 0, cutblk)
        stage_zone_a(h_stage[0], hown[0], h_fullA[0])
        transform_blocks(xT, W1s, hown[0], cutblk, nblk)
        stage_zone_b(h_stage[0], hown[0], h_fullB[0])

        def fin1(b, py):
            ys = y1T[:, b * 128 : (b + 1) * 128]
            nc.vector.tensor_tensor(
                out=ys, in0=py[:], in1=dinvb[:, b * 128 : (b + 1) * 128], op=mul
            )
            nc.scalar.activation(ys, ys, RELU, bias=b1s[:, 0:1])

        def l2_head():
            # emitted mid-way through layer-1 gathers: transform the zone-A
            # half of layer 2 and kick its AllGather while L1 still gathers
            transform_blocks(y1T, W2s, hown[1], 0, cutblk)
            stage_zone_a(h_stage[1], hown[1], h_fullA[1])

        gather_reduce(
            h_fullA[0], h_fullB[0], fin1, hown[0], mid_cb=l2_head, mid_at=35
        )

        # ---- layer 2 -----------------------------------------------------
        transform_blocks(y1T, W2s, hown[1], cutblk, nblk)
        stage_zone_b(h_stage[1], hown[1], h_fullB[1])

        def fin2(b, py):
            yt = outp.tile([128, F], f32)
            nc.vector.tensor_tensor(
                out=yt[:], in0=py[:], in1=dinvb[:, b * 128 : (b + 1) * 128], op=mul
            )
            r = 128 if b < nblk - 1 else last_rows
            nc.sync.dma_start(yout[:, b * 128 : b * 128 + r], yt[:, :r])

        gather_reduce(h_fullA[1], h_fullB[1], fin2, hown[1])

    nc.compile()
    return nc


def get_program(cfg, sched):
    key = (tuple(sorted(cfg.items())), sched)
    if key not in _PROGRAM_CACHE:
        _PROGRAM_CACHE[key] = build_program(cfg, sched)
    return _PROGRAM_CACHE[key]


# ----------------------------------------------------------------------------
# input marshalling + entry point
# ----------------------------------------------------------------------------


def make_in_maps(x, W1, b1, W2, b2, cfg, per_core, deg):
    N, F, ncores, rows, nblk = (
        cfg["N"],
        cfg["F"],
        cfg["ncores"],
        cfg["rows"],
        cfg["nblk"],
    )
    import ml_dtypes

    td = ml_dtypes.bfloat16 if cfg["table_bf16"] else np.float32
    x = np.asarray(x, np.float32)
    W1 = np.ascontiguousarray(np.asarray(W1, np.float32)).astype(td)
    W2 = np.ascontiguousarray(np.asarray(W2, np.float32)).astype(td)
    b1_col = np.ascontiguousarray(np.asarray(b1, np.float32).reshape(128, 1))
    iota = np.ascontiguousarray(
        np.broadcast_to(np.arange(128, dtype=np.float32), (128, 128))
    ).astype(td)
    ident = np.eye(128, dtype=np.float32).astype(td)
    dinv = (1.0 / np.sqrt(deg.astype(np.float64))).astype(np.float32)
    in_maps = []
    for k in range(ncores):
        xk = np.zeros((nblk * 128, F), np.float32)
        xk[:rows] = x[k * rows : (k + 1) * rows]
        dk = np.ones(nblk * 128, np.float32)
        dk[:rows] = dinv[k * rows : (k + 1) * rows]
        dinv_row = np.ascontiguousarray(dk.reshape(nblk, 128).T)
        dinv_bc = np.ascontiguousarray(
            np.broadcast_to(dk[None, :], (128, nblk * 128))
        ).astype(td)
        pc = per_core[k]
        in_maps.append(
            dict(
                x_own=xk.astype(td),
                W1=W1,
                W2=W2,
                b1_col=b1_col,
                dinv_row=dinv_row,
                dinv_bc=dinv_bc,
                iota=iota,
                ident=ident,
                idx_lo=pc["idx_lo"],
                idx_hi=pc["idx_hi"],
                dl_lo=pc["dl_lo"].astype(td),
                dl_hi=pc["dl_hi"].astype(td),
            )
        )
    return in_maps


def _ensure_ntff_hook():
    """Register the NTFF profiling hook (missing antenv.axon_hooks shim)."""
    try:
        from antenv.axon_hooks import get_axon_ntff_profile_hook  # noqa: F401

        return True
    except ImportError:
        pass
    try:
        import sys
        import types

        import antenv
        from trn_agent_boot.trn_boot import _ntff_profile_via_ctypes

        hook = _ntff_profile_via_ctypes("/opt/axon/libaxon_pjrt.so")
        if hook is None:
            return False
        mod = types.ModuleType("antenv.axon_hooks")
        mod._hook = hook
        mod.get_axon_ntff_profile_hook = lambda: mod._hook
        mod.set_axon_ntff_profile_hook = lambda h: setattr(mod, "_hook", h)
        sys.modules["antenv.axon_hooks"] = mod
        antenv.axon_hooks = mod
        # artifact upload needs cloud credentials; stub it out
        import concourse.bass_utils as bu

        bu.upload_artifacts = lambda tmpdir: f"local:{tmpdir}"
        return True
    except Exception:
        return False


def run(x, edge_index, W1, b1, W2, b2, cfg, trace=False):
    from concourse.bass_utils import run_bass_kernel_spmd

    if trace:
        trace = _ensure_ntff_hook()

    sched, per_core, deg = preprocess(edge_index, cfg)
    nc = get_program(cfg, sched)
    in_maps = make_in_maps(x, W1, b1, W2, b2, cfg, per_core, deg)
    res = run_bass_kernel_spmd(
        nc, in_maps, list(range(cfg["ncores"])), trace=trace
    )
    b2f = np.asarray(b2, np.float32)
    out = np.concatenate(
        [res.results[k]["y_out"].T for k in range(cfg["ncores"])], axis=0
    ) + b2f[None, :]
    return out.astype(np.float32), res


def kernel(x, edge_index, W1, b1, W2, b2):
    out, _ = run(x, edge_index, W1, b1, W2, b2, FULL_CFG)
    return out
